# revision 21
# baseline (speedup 1.0000x reference)
"""Trainium2 Bass kernel for nn_ExHuneLSTM (bidirectional single-step LSTM scan).

Key observation: the forward-direction cell is DEAD CODE in the reference.
x_hat = lstm_out[:, 0, -1:] takes the LAST channel of concat(h_fwd, h_bwd),
i.e. h_bwd[:, H-1]; the module output is u + 0.5*x_hat_prime + 0.5*x_hat_next,
built only from u and the two backward-cell scalars. h_fwd/c_fwd feed nothing
else. So only the backward-parameter cell is computed here.

Math per timestep (b-cell only, run twice):
  * phase A: gates1 = P_b[t] + W_hh_b h + (bias), update (h, c); xh = h[:,511]
  * phase B: gates2 = P_b[t] + W_hh_b h + xh * rowsum(w_ih_b) + (bias)
    (input x_hat_prime = u_t + xh is a rank-1 correction on the projection)
  * out[b,t,:] = 1.5*u[b,t,:] + 0.5*(xh + xhn)
  * P_b[t] = u_t @ w_ih_b.T + (b_ih_b + b_hh_b) precomputed with a big GEMM.

Layout per core (batch sharded 8 ways, B_local=4):
  * Gates in PSUM (128, 512): partition 32q+b (q = H-quarter, b = batch),
    free = [i|f|g|o] x 128 per quarter; g-gate weights pre-scaled 2x so
    tanh(g) = 2*sigmoid(2g)-1 comes from the one fused sigmoid.
  * Recurrent matmul: stationary = hT column slices (128, 4); moving = w_hh.T
    chunks (128, 512) bf16 through 4 PE column groups (tile_position).
  * Cell state kept as d = c/2 in bf16:
      d' = sig_f*d + (sig_2g - 0.5)*sig_i   (one STT + two TT)
      tanh(c) = ACT(d, Tanh, scale=2)
  * Sigmoid split [i|f|g] / [o] so the c-chain starts one ACT earlier.
  * P injected via a K=16 selector matmul, pre-issued one phase ahead.
  * Junk filler matmuls + a pre-loop burst keep the PE HAM clock at 2.4 GHz
    (bursty PE work otherwise throttles to 1.2 GHz).
"""

import math
import os
from contextlib import ExitStack

import numpy as np
import ml_dtypes

import concourse.bass as bass
import concourse.mybir as mybir
import concourse.tile as tile
from concourse import bacc
from concourse.bass import ds
from concourse.bass_utils import run_bass_kernel_spmd

F32 = mybir.dt.float32
BF16 = mybir.dt.bfloat16
AF = mybir.ActivationFunctionType
OP = mybir.AluOpType

B, D, H = 32, 512, 512
G4 = 4 * H  # 2048 gate dim
N_CORES = 8
BL = B // N_CORES  # 4 batch rows per core

_BF = ml_dtypes.bfloat16

# keep-warm filler configuration (sized for the WARM steady-state windows;
# the HAM clock gate is bistable, so per-body bursts force the warm state)
FILL_SIG = 2          # no-dep fillers (N=512) covering the sig_hi window
FILL_DVE = 5          # sig_hi-gated fillers (N=512) covering the dve/tanh window
FILL_TAIL = 2         # fillers (N=128) covering the hT-copy window
WARMUP_MMS = 30       # dense junk matmuls before the loop
BODY_BURST = 8        # dense junk matmuls at each loop-body top (re-warm
                      # after the back-edge barrier, hidden under the slab DMA)
UNROLL = 16


# ---------------------------------------------------------------- host prep --

def _perm_rows(x):
    """Permute gate-dim (4H) from gate-major (g,q,h) to quarter-major
    (q,g,h) along axis 0."""
    s = x.shape
    y = x.reshape(4, 4, 128, *s[1:])
    return y.transpose(1, 0, 2, *range(3, y.ndim)).reshape(*s)


def _prep_dir(w_ih, w_hh, b_ih, b_hh):
    """Returns (whh_img (128, 8192) bf16, wih_img (128, 8192) bf16,
    bias (1, 2048) f32, rowsum (1, 2048) bf16) with gate permutation and
    2x pre-scale on the g gate."""
    w_ih = np.asarray(w_ih, np.float32).copy()
    w_hh = np.asarray(w_hh, np.float32).copy()
    bias = (np.asarray(b_ih, np.float32) + np.asarray(b_hh, np.float32)).copy()
    rowsum = w_ih.sum(axis=1)
    # 2x scale on g gate (PyTorch order i,f,g,o -> rows 1024:1536)
    w_ih[1024:1536] *= 2.0
    w_hh[1024:1536] *= 2.0
    bias = bias.copy(); bias[1024:1536] *= 2.0
    rowsum = rowsum.copy(); rowsum[1024:1536] *= 2.0
    w_ih = _perm_rows(w_ih)
    w_hh = _perm_rows(w_hh)
    bias = _perm_rows(bias)
    rowsum = _perm_rows(rowsum)

    def img(w):  # (2048, 512) -> w.T (512, 2048) -> K-chunks side by side
        wt = w.T.astype(_BF)                       # (512, 2048)
        return np.ascontiguousarray(
            wt.reshape(4, 128, G4).transpose(1, 0, 2).reshape(128, 4 * G4))

    return (img(w_hh), img(w_ih),
            bias.reshape(1, G4).astype(np.float32),
            rowsum.reshape(1, G4).astype(_BF))


# ------------------------------------------------------------ device program --

def build_program(T, unroll=UNROLL, use_loop=True):
    nc = bacc.Bacc("TRN2", num_devices=N_CORES, debug=False)

    u_d = nc.dram_tensor("u", (BL, T, D), F32, kind="ExternalInput")
    whhb_d = nc.dram_tensor("whh_b", (128, 4 * G4), BF16, kind="ExternalInput")
    wihb_d = nc.dram_tensor("wih_b", (128, 4 * G4), BF16, kind="ExternalInput")
    biasb_d = nc.dram_tensor("bias_b", (1, G4), F32, kind="ExternalInput")
    rsb_d = nc.dram_tensor("rs_b", (1, G4), BF16, kind="ExternalInput")
    ident_d = nc.dram_tensor("ident", (128, 128), BF16, kind="ExternalInput")
    s16_d = nc.dram_tensor("s16", (16, 128), BF16, kind="ExternalInput")
    i4rep_d = nc.dram_tensor("i4rep", (128, 4), BF16, kind="ExternalInput")
    out_d = nc.dram_tensor("out", (BL, T, D), F32, kind="ExternalOutput")

    pb_d = nc.dram_tensor("pb_scratch", (T, 16, 512), BF16, kind="Internal")
    s_d = nc.dram_tensor("s_scratch", (BL * T,), F32, kind="Internal")

    ntok = BL * T
    nchunk = math.ceil(ntok / 128)

    with ExitStack() as ctx:
        tc = ctx.enter_context(tile.TileContext(nc))

        consts = ctx.enter_context(tc.tile_pool(name="consts", bufs=1))
        wpool = ctx.enter_context(tc.tile_pool(name="weights", bufs=1))

        ident_t = consts.tile([128, 128], BF16)
        nc.sync.dma_start(out=ident_t, in_=ident_d.ap())
        s16_t = consts.tile([16, 128], BF16)
        nc.sync.dma_start(out=s16_t, in_=s16_d.ap())
        i4rep_t = consts.tile([128, 4], BF16)
        nc.sync.dma_start(out=i4rep_t, in_=i4rep_d.ap())
        rsb_t = consts.tile([1, G4], BF16)
        nc.sync.dma_start(out=rsb_t, in_=rsb_d.ap())

        def bcast128(dram_handle):
            a = dram_handle.ap()
            return bass.AP(tensor=a.tensor, offset=a.offset,
                           ap=[[0, 128], list(a.ap[-1])])

        whhb_t = wpool.tile([128, 4 * G4], BF16)
        nc.sync.dma_start(out=whhb_t, in_=whhb_d.ap())

        u_flat = u_d.ap().rearrange("b t d -> (b t) d")
        # ---------------- precompute P_b = u @ wih_b.T + bias ---------------
        with tc.tile_pool(name="pre_sb", bufs=3) as pre_sb, \
             tc.tile_pool(name="pre_w", bufs=1) as pre_w, \
             tc.tile_pool(name="pre_ps", bufs=2, space="PSUM") as pre_ps, \
             tc.tile_pool(name="pre_gps", bufs=1, space="PSUM") as pre_gps:
            wihb_t = pre_w.tile([128, 4 * G4], BF16)
            nc.sync.dma_start(out=wihb_t, in_=wihb_d.ap())
            biasb_t = pre_w.tile([128, G4], F32)
            nc.sync.dma_start(out=biasb_t, in_=bcast128(biasb_d))

            # (T, 16, 512): row 4q+b holds P'[t, b, quarter q]
            pb_store = pb_d.ap().rearrange("t (q b) n -> t q b n", b=BL)

            def token_ranges(m):
                lo = 128 * m
                hi = min(lo + 128, ntok)
                t0g = lo
                while t0g < hi:
                    b = t0g // T
                    t1g = min(hi, (b + 1) * T)
                    yield t0g - lo, b, t0g - b * T, t1g - b * T
                    t0g = t1g
            for m in range(nchunk):
                P = min(128, ntok - 128 * m)
                u_sb = pre_sb.tile([128, D], F32, tag="u_raw")
                nc.sync.dma_start(out=u_sb[:P], in_=u_flat[128 * m:128 * m + P, :])
                u_bf = pre_sb.tile([128, D], BF16, tag="u_bf")
                nc.vector.tensor_copy(u_bf[:P], u_sb[:P])
                ut_ps = pre_ps.tile([128, 512], BF16, tag="ut_ps")
                for k in range(4):
                    nc.tensor.transpose(ut_ps[:, 128 * k:128 * k + P],
                                        u_bf[:P, 128 * k:128 * (k + 1)],
                                        ident_t[:P, :P])
                ut_sb = pre_sb.tile([128, 512], BF16, tag="ut_sb")
                nc.vector.tensor_copy(ut_sb, ut_ps)
                g_ps = pre_gps.tile([128, G4], F32, tag="pre_g")
                for k in range(4):
                    for n in range(4):
                        nc.tensor.matmul(
                            g_ps[:P, 512 * n:512 * (n + 1)],
                            ut_sb[:, 128 * k:128 * k + P],
                            wihb_t[:, G4 * k + 512 * n:G4 * k + 512 * (n + 1)],
                            start=(k == 0), stop=(k == 3))
                g_bf = pre_sb.tile([128, G4], BF16, tag="pre_o")
                nc.vector.scalar_tensor_tensor(
                    g_bf[:P], g_ps[:P], 1.0, biasb_t[:P],
                    op0=OP.mult, op1=OP.add)
                for r0, b_, t0, t1 in token_ranges(m):
                    nc.sync.dma_start(
                        out=pb_store[t0:t1, :, b_, :],
                        in_=g_bf[r0:r0 + (t1 - t0)].rearrange(
                            "r (q n) -> r q n", q=4))

        # ---------------- recurrence ----------------------------------------
        state = ctx.enter_context(tc.tile_pool(name="state", bufs=1))
        hbT = state.tile([128, 128], BF16)
        db = state.tile([128, 128], BF16)       # d = c/2
        xh_al = state.tile([1, 4], BF16)
        xh_s = state.tile([1, ntok], F32)
        xhn_s = state.tile([1, ntok], F32)
        for t_ in (hbT, db, xh_al):
            nc.vector.memset(t_, 0.0)

        warm = consts.tile([1, 4], BF16)
        nc.scalar.activation(warm, xh_al[0:1, 0:4], AF.Sigmoid)
        nc.scalar.activation(warm, xh_al[0:1, 0:4], AF.Tanh)
        ppool = ctx.enter_context(tc.tile_pool(name="prefetch", bufs=2))
        # PSUM budget (8 banks, bank-granular slots):
        # Gl x2 + Gh x2 + hTp x2 (shared with xh_ps) + junk x1 = 7
        gps = ctx.enter_context(tc.tile_pool(name="gates_ps", bufs=2, space="PSUM"))
        hps = ctx.enter_context(tc.tile_pool(name="ht_ps", bufs=2, space="PSUM"))
        jpool = ctx.enter_context(tc.tile_pool(name="junk_ps", bufs=1, space="PSUM"))
        work = ctx.enter_context(tc.tile_pool(name="work", bufs=3))

        junk_ps = jpool.tile([128, 512], F32)

        def filler(src, n, w):
            """Junk matmuls that keep the PE HAM-warm through idle windows.
            src gates when they run (they read it); junk_ps swallows output."""
            for _ in range(n):
                nc.tensor.matmul(junk_ps[0:4, 0:w], i4rep_t[:, 0:4],
                                 src[:, 0:w], start=True, stop=True)

        def new_G(p_t):
            """Allocate the two gate-half PSUM tiles ([i|f] and [g|o]) in
            separate banks (start=True clears per bank) and pre-issue the
            P injections."""
            Gl = gps.tile([128, 256], F32, tag="Gl")
            Gh = gps.tile([128, 256], F32, tag="Gh")
            nc.tensor.matmul(Gl, s16_t, p_t[:, 0:256], start=True, stop=False)
            nc.tensor.matmul(Gh, s16_t, p_t[:, 256:512], start=True, stop=False)
            return (Gl, Gh)

        def half_rounds(Gx, off, use_xh):
            """Accumulate W_hh h (+ optional xh*rs) into one 256-col half.
            off = 0 for [i|f], 256 for [g|o]."""
            for k in range(4):
                for j in range(4):
                    c0 = G4 * k + 512 * j + off
                    nc.tensor.matmul(
                        Gx[32 * j:32 * j + 4, :], hbT[:, 32 * k:32 * k + 4],
                        whhb_t[:, c0:c0 + 256],
                        start=False,
                        stop=(not use_xh) and k == 3 and j == 3,
                        tile_position=(0, 32 * j))
            if use_xh:
                for j in range(4):
                    c0 = 512 * j + off
                    nc.tensor.matmul(Gx[32 * j:32 * j + 4, :], xh_al,
                                     rsb_t[0:1, c0:c0 + 256],
                                     start=False, stop=(j == 3),
                                     tile_position=(0, 32 * j))

        def phase(G, p_next, use_xh, hist_slice, to_al):
            """One b-cell evaluation. G=(Gl,Gh) is already injected with P.
            Returns the next phase's (pre-injected) G, or None."""
            Gl, Gh = G
            half_rounds(Gl, 0, use_xh)      # [i|f]
            sig_lo = work.tile([128, 256], BF16, tag="sigl")
            nc.scalar.activation(sig_lo, Gl, AF.Sigmoid)
            half_rounds(Gh, 256, use_xh)    # [g|o], overlaps sig_lo on ACT
            sig_hi = work.tile([128, 256], BF16, tag="sigh")
            nc.scalar.activation(sig_hi, Gh, AF.Sigmoid)
            # d' = sig_f*d + (sig_2g - 0.5)*sig_i ;  tanh(c)=tanh(2d)
            t1 = work.tile([128, 128], BF16, tag="t1")
            nc.vector.tensor_tensor(t1, sig_lo[:, 128:256], db, OP.mult)
            t2 = work.tile([128, 128], BF16, tag="t2")
            nc.vector.scalar_tensor_tensor(t2, sig_hi[:, 0:128], 0.5,
                                           sig_lo[:, 0:128],
                                           op0=OP.subtract, op1=OP.mult)
            nc.vector.tensor_tensor(db, t1, t2, OP.add)
            tc2 = work.tile([128, 128], BF16, tag="tc")
            nc.scalar.activation(tc2, db, AF.Tanh, scale=2.0)
            h2 = work.tile([128, 128], BF16, tag="h2")
            nc.vector.tensor_tensor(h2, sig_hi[:, 128:256], tc2, OP.mult)
            filler(whhb_t, FILL_SIG, 512)   # no-dep: runs during sig_hi
            filler(sig_hi, FILL_DVE, 256)   # gated: runs during dve/tanh
            # hT for the next recurrent matmul (chain-critical: first)
            hT_ps = hps.tile([128, 128], BF16, tag="hTp")
            nc.tensor.transpose(hT_ps, h2, ident_t)
            # xh scalar extraction (h[:,511] lives at h2[96+b, 127])
            xh_ps = hps.tile([1, 4], BF16, tag="hTp")
            nc.tensor.transpose(xh_ps, h2[96:100, 127:128],
                                i4rep_t[96:100, :], tile_position=(96, 0))
            filler(tc2, FILL_TAIL, 128)
            G2 = new_G(p_next) if p_next is not None else None
            nc.vector.tensor_copy(hbT, hT_ps)
            if to_al:
                nc.vector.tensor_copy(xh_al, xh_ps)
            nc.vector.tensor_copy(hist_slice, xh_ps)
            return G2

        def step(t_expr, G, p_this, p_next):
            """Two phases; returns pre-injected G for the next step (or None).
            p_this re-injects for phase B; p_next pre-injects the next step."""
            G = phase(G, p_this, False, xh_s[0:1, ds(t_expr * 4, 4)], True)
            return phase(G, p_next, True, xhn_s[0:1, ds(t_expr * 4, 4)], False)

        # dense junk burst to flip the HAM clock gate to 8/8 before the loop
        for w_i in range(WARMUP_MMS):
            nc.tensor.matmul(junk_ps[0:4, :], i4rep_t[:, 0:4],
                             whhb_t[:, 0:512], start=True, stop=True)

        if use_loop:
            assert T % unroll == 0
            with tc.For_i(0, T // unroll, 1,
                          hint_engines=tuple(mybir.ALL_ENGINES)) as i:
                slab_b = ppool.tile([16, 512 * unroll], BF16, tag="slabb")
                nc.sync.dma_start(
                    out=slab_b,
                    in_=pb_d.ap()[ds(i * unroll, unroll)].transpose([1, 0, 2]))
                # re-warm the PE after the back-edge barrier, under the DMA
                filler(whhb_t, BODY_BURST, 512)
                pcol = lambda s: slab_b[:, 512 * s:512 * (s + 1)]
                G = new_G(pcol(0))
                for s_ in range(unroll):
                    p_next = pcol(s_ + 1) if s_ + 1 < unroll else None
                    G = step(i * unroll + s_, G, pcol(s_), p_next)
        else:
            with tc.tile_pool(name="pstep", bufs=2) as pstep:
                G = None
                for t_ in range(T):
                    pb_t = pstep.tile([16, 512], BF16, tag="pb")
                    nc.sync.dma_start(out=pb_t,
                                      in_=pb_d.ap()[ds(t_, 1)].squeeze(0))
                    if G is None:
                        G = new_G(pb_t)
                    G = step(t_, G, pb_t, None)

        # ---------------- output pass ---------------------------------------
        nc.vector.tensor_tensor(xh_s, xh_s, xhn_s, OP.add)
        nc.vector.tensor_scalar(xh_s, xh_s, 0.5, None, OP.mult)
        nc.sync.dma_start(out=s_d.ap().unsqueeze(0), in_=xh_s[0:1, :])
        s_bm = s_d.ap().rearrange("(t b) -> t b", b=BL).transpose([1, 0])
        out_flat = out_d.ap().rearrange("b t d -> (b t) d")

        def token_ranges2(m):
            lo, hi = 128 * m, min(128 * m + 128, ntok)
            t0g = lo
            while t0g < hi:
                b = t0g // T
                t1g = min(hi, (b + 1) * T)
                yield t0g - lo, b, t0g - b * T, t1g - b * T
                t0g = t1g

        with tc.tile_pool(name="post", bufs=4) as post:
            for m in range(nchunk):
                P = min(128, ntok - 128 * m)
                u_sb = post.tile([128, D], F32, tag="u_post")
                nc.sync.dma_start(out=u_sb[:P], in_=u_flat[128 * m:128 * m + P, :])
                s_pp = post.tile([128, 1], F32, tag="s_pp")
                for r0, b_, t0, t1 in token_ranges2(m):
                    nc.sync.dma_start(
                        out=s_pp[r0:r0 + (t1 - t0)],
                        in_=s_bm[b_, t0:t1].unsqueeze(1))
                o_sb = post.tile([128, D], F32, tag="o_post")
                nc.vector.tensor_scalar(o_sb[:P], u_sb[:P], 1.5, s_pp[:P],
                                        OP.mult, OP.add)
                nc.sync.dma_start(out=out_flat[128 * m:128 * m + P, :],
                                  in_=o_sb[:P])

    nc.finalize()
    return nc


# ------------------------------------------------------------------- runner --

_CACHE = {}


def _get_program(T, unroll, use_loop):
    key = (T, unroll, use_loop)
    if key not in _CACHE:
        _CACHE[key] = build_program(T, unroll=unroll, use_loop=use_loop)
    return _CACHE[key]


def _run_pjrt(nc, in_maps, time_iters=0):
    """Execute via PJRT shard_map (like bass2jax.run_bass_via_pjrt) but keep
    the jitted callable so repeated timed executions reuse staged inputs.
    Returns (results_list, best_ns or None)."""
    import time as _time
    import jax
    from jax.sharding import Mesh, PartitionSpec
    from jax.experimental.shard_map import shard_map
    import concourse.mybir as _mb
    from concourse import bass2jax as b2j

    b2j.install_neuronx_cc_hook()
    n_cores = len(in_maps)
    partition_name = nc.partition_id_tensor.name if nc.partition_id_tensor else None
    in_names, out_names, out_avals, zero_outs = [], [], [], []
    for alloc in nc.m.functions[0].allocations:
        if not isinstance(alloc, _mb.MemoryLocationSet):
            continue
        name = alloc.memorylocations[0].name
        if alloc.kind == "ExternalInput":
            if name != partition_name:
                in_names.append(name)
        elif alloc.kind == "ExternalOutput":
            shape = tuple(alloc.tensor_shape)
            dtype = _mb.dt.np(alloc.dtype)
            out_names.append(name)
            out_avals.append(jax.core.ShapedArray(shape, dtype))
            zero_outs.append(np.zeros(shape, dtype))
    n_params = len(in_names)
    all_in = list(in_names) + list(out_names)
    if partition_name is not None:
        all_in.append(partition_name)

    def _body(*args):
        operands = list(args)
        if partition_name is not None:
            operands.append(b2j.partition_id_tensor())
        outs = b2j._bass_exec_p.bind(
            *operands, out_avals=tuple(out_avals), in_names=tuple(all_in),
            out_names=tuple(out_names), lowering_input_output_aliases=(),
            sim_require_finite=True, sim_require_nnan=True, nc=nc)
        return tuple(outs)

    devices = jax.devices()[:n_cores]
    mesh = Mesh(np.array(devices), ("core",))
    n_outs = len(out_names)
    sharded = jax.jit(
        shard_map(_body, mesh=mesh,
                  in_specs=(PartitionSpec("core"),) * (n_params + n_outs),
                  out_specs=(PartitionSpec("core"),) * n_outs,
                  check_rep=False),
        keep_unused=True)
    concat_in = [np.concatenate([np.asarray(in_maps[c][nm])
                                 for c in range(n_cores)], axis=0)
                 for nm in in_names]
    concat_zeros = [np.zeros((n_cores * z.shape[0], *z.shape[1:]), z.dtype)
                    for z in zero_outs]
    args = [jax.device_put(a) for a in concat_in + concat_zeros]
    out_arrs = jax.block_until_ready(sharded(*args))
    best = None
    for _ in range(time_iters):
        t0 = _time.perf_counter()
        out_arrs2 = jax.block_until_ready(sharded(*args))
        dt = _time.perf_counter() - t0
        best = dt if best is None or dt < best else best
    results = [{nm: np.asarray(out_arrs[i]).reshape(n_cores,
                                                    *out_avals[i].shape)[c]
                for i, nm in enumerate(out_names)}
               for c in range(n_cores)]
    return results, (None if best is None else int(best * 1e9))


def kernel(u_sequence, w_ih_f, w_hh_f, b_ih_f, b_hh_f,
           w_ih_b, w_hh_b, b_ih_b, b_hh_b, _trace=False, _time_iters=0):
    u = np.asarray(u_sequence, np.float32)
    Bn, T, Dn = u.shape
    assert (Bn, Dn) == (B, D)

    whhb_i, wihb_i, biasb, rsb = _prep_dir(w_ih_b, w_hh_b, b_ih_b, b_hh_b)
    ident = np.eye(128, dtype=_BF)
    s16 = np.zeros((16, 128), dtype=_BF)
    for q in range(4):
        for b_ in range(BL):
            s16[4 * q + b_, 32 * q + b_] = 1
    i4rep = np.zeros((128, 4), dtype=_BF)
    for p in range(128):
        if p % 32 < 4:
            i4rep[p, p % 32] = 1

    unroll = UNROLL
    use_loop = (T % unroll == 0) and T >= unroll \
        and not os.environ.get('KERNEL_NO_LOOP')
    nc = _get_program(T, unroll, use_loop)

    common = dict(whh_b=whhb_i, wih_b=wihb_i, bias_b=biasb, rs_b=rsb,
                  ident=ident, s16=s16, i4rep=i4rep)
    in_maps = []
    for c in range(N_CORES):
        m = dict(common)
        m["u"] = np.ascontiguousarray(u[c * BL:(c + 1) * BL])
        in_maps.append(m)

    results, best_ns = _run_pjrt(nc, in_maps, time_iters=_time_iters)
    out = np.concatenate([results[c]["out"] for c in range(N_CORES)], axis=0)
    kernel._last_ns = best_ns
    return out


if __name__ == "__main__":
    # tiny smoke: T=16 vs jax reference
    import reference
    T_s = 16
    inputs = reference.setup_inputs()
    inputs = {k: np.asarray(v) for k, v in inputs.items()}
    inputs["u_sequence"] = inputs["u_sequence"][:, :T_s, :]
    want = np.asarray(reference.reference(**inputs))
    got = kernel(**inputs)
    err = np.abs(got - want)
    rel = err.max() / np.abs(want).max()
    print(f"T={T_s}  maxabs={err.max():.3e}  rel={rel:.3e}")
